# revision 1
# baseline (speedup 1.0000x reference)
"""Pair-packed variant: lanes sorted by src; same-row edge pairs share one
src gather ([P,D] fetch serves two compute slots). Phase A = pairs,
phase C = singles. Host unpermutes outputs via orig-id map."""

import numpy as np

import concourse.bass as bass
import concourse.mybir as mybir
import concourse.tile as tile
from concourse import bacc
from concourse.bass import IndirectOffsetOnAxis
from concourse.bass_utils import run_bass_kernel_spmd
from concourse.masks import make_identity
from contextlib import ExitStack

N, D, H = 100000, 128, 128
E_TOTAL = 2000000
NCORES = 8
P = 128
BLK_SLOTS = 4
REG_SUP = 16  # superslots per phase-A region (32 slots)
REG_C = 32  # slots per phase-C region
E_CORE = E_TOTAL // NCORES
S_LANE = -(-E_CORE // P)  # 1954 edges per lane (padded)

F32 = mybir.dt.float32
I32 = mybir.dt.int32
RELU = mybir.ActivationFunctionType.Relu
IDENT = mybir.ActivationFunctionType.Identity


def _block(nc, work, psum_t, psum_h, psum_o, ident, w1_sb, b1_sb, w2_sb, b2_sb,
           ef_srcs, o_stage, e0):
    """Shared 4-slot (512-edge) compute block. ef_srcs: 4 (zs_ap, zd_ap) pairs."""
    EB = BLK_SLOTS * P
    ef = work.tile([P, EB], F32, tag="ef")
    for c, (a, b) in enumerate(ef_srcs):
        nc.vector.tensor_mul(out=ef[:, c * P : (c + 1) * P], in0=a, in1=b)
    efT_ps = psum_t.tile([P, EB], F32)
    for c in range(BLK_SLOTS):
        nc.tensor.transpose(
            out=efT_ps[:, c * P : (c + 1) * P],
            in_=ef[:, c * P : (c + 1) * P],
            identity=ident[:],
        )
    efT = work.tile([P, EB], F32, tag="efT")
    nc.vector.tensor_copy(out=efT[:], in_=efT_ps[:])
    h_ps = psum_h.tile([P, EB], F32)
    nc.tensor.matmul(out=h_ps[:], lhsT=w1_sb[:], rhs=efT[:], start=True, stop=True)
    h_sb = work.tile([P, EB], F32, tag="h")
    nc.scalar.activation(out=h_sb[:], in_=h_ps[:], func=RELU, bias=b1_sb[:, :1],
                         scale=1.0)
    o_ps = psum_o.tile([1, EB], F32)
    nc.tensor.matmul(out=o_ps[:], lhsT=w2_sb[:], rhs=h_sb[:], start=True, stop=True)
    nc.scalar.activation(
        out=o_stage[:1, e0 : e0 + EB], in_=o_ps[:], func=IDENT,
        bias=b2_sb[:1, :1], scale=1.0,
    )


def build_program(nA, nC, n=N):
    nc = bacc.Bacc("TRN2", target_bir_lowering=False, debug=False,
                   enable_asserts=False, num_devices=NCORES)
    X = 3 * nA + 2 * nC
    z_d = nc.dram_tensor("z", [n, D], F32, kind="ExternalInput").ap()
    idx_d = nc.dram_tensor("idx", [P, X], I32, kind="ExternalInput").ap()
    w1_d = nc.dram_tensor("w1", [D, H], F32, kind="ExternalInput").ap()
    b1_d = nc.dram_tensor("b1", [H], F32, kind="ExternalInput").ap()
    w2_d = nc.dram_tensor("w2", [H, 1], F32, kind="ExternalInput").ap()
    b2_d = nc.dram_tensor("b2", [1], F32, kind="ExternalInput").ap()
    out_d = nc.dram_tensor("out", [(2 * nA + nC) * P], F32,
                           kind="ExternalOutput").ap()

    with tile.TileContext(nc) as tc, ExitStack() as ctx:
        const = ctx.enter_context(tc.tile_pool(name="const", bufs=1))
        zpool = ctx.enter_context(tc.tile_pool(name="gather", bufs=3))
        work = ctx.enter_context(tc.tile_pool(name="work", bufs=3))
        stage_pool = ctx.enter_context(tc.tile_pool(name="stage", bufs=2))
        psum_t = ctx.enter_context(tc.tile_pool(name="ps_t", bufs=2, space="PSUM"))
        psum_h = ctx.enter_context(tc.tile_pool(name="ps_h", bufs=2, space="PSUM"))
        psum_o = ctx.enter_context(tc.tile_pool(name="ps_o", bufs=2, space="PSUM"))

        idx_sb = const.tile([P, X], I32)
        nc.sync.dma_start(out=idx_sb[:], in_=idx_d[:, :])
        w1_sb = const.tile([P, H], F32)
        nc.sync.dma_start(out=w1_sb[:], in_=w1_d[:, :])
        b1_sb = const.tile([P, 1], F32)
        nc.sync.dma_start(out=b1_sb[:], in_=b1_d[:, None])
        w2_sb = const.tile([P, 1], F32)
        nc.sync.dma_start(out=w2_sb[:], in_=w2_d[:, :])
        b2_sb = const.tile([1, 1], F32)
        nc.sync.dma_start(out=b2_sb[:1], in_=b2_d[:, None])
        ident = const.tile([P, P], F32)
        make_identity(nc, ident[:])

        def gather(dst_ap, col0):
            nc.gpsimd.indirect_dma_start(
                out=dst_ap, out_offset=None, in_=z_d[:, :],
                in_offset=IndirectOffsetOnAxis(ap=idx_sb[:, col0 : col0 + 1],
                                               axis=0),
            )

        blk = (nc, work, psum_t, psum_h, psum_o, ident, w1_sb, b1_sb, w2_sb, b2_sb)

        # ---- phase A: paired slots (one src gather serves two slots) ----
        for g in range(-(-nA // REG_SUP)):
            t0 = g * REG_SUP
            gsup = min(REG_SUP, nA - t0)
            zs_t = zpool.tile([P, REG_SUP * D], F32, tag="zs")
            zd_t = zpool.tile([P, 2 * REG_SUP * D], F32, tag="zd")
            for t in range(gsup):
                gather(zs_t[:, t * D : (t + 1) * D], t0 + t)
                gather(zd_t[:, (2 * t) * D : (2 * t + 1) * D], nA + t0 + t)
                gather(zd_t[:, (2 * t + 1) * D : (2 * t + 2) * D],
                       2 * nA + t0 + t)
            o_stage = stage_pool.tile([1, 2 * REG_SUP * P], F32, tag="ostage")
            for b in range(gsup * 2 // BLK_SLOTS):
                srcs = []
                for c in range(BLK_SLOTS):
                    s = b * BLK_SLOTS + c
                    w = s // 2
                    srcs.append((zs_t[:, w * D : (w + 1) * D],
                                 zd_t[:, s * D : (s + 1) * D]))
                _block(*blk, srcs, o_stage, b * BLK_SLOTS * P)
            nc.sync.dma_start(
                out=out_d[(2 * t0) * P : (2 * t0 + 2 * gsup) * P][None, :],
                in_=o_stage[:1, : 2 * gsup * P],
            )

        # ---- phase C: single slots ----
        cbase = 3 * nA
        obase = 2 * nA
        for g in range(-(-nC // REG_C)):
            s0 = g * REG_C
            gslots = min(REG_C, nC - s0)
            zc_t = zpool.tile([P, 2 * REG_SUP * D], F32, tag="zd")
            zdc_t = zpool.tile([P, REG_SUP * D] if False else [P, 2 * REG_SUP * D],
                               F32, tag="zs2")
            for k in range(gslots):
                gather(zc_t[:, k * D : (k + 1) * D], cbase + s0 + k)
                gather(zdc_t[:, k * D : (k + 1) * D], cbase + nC + s0 + k)
            o_stage = stage_pool.tile([1, 2 * REG_SUP * P], F32, tag="ostage")
            for b in range(gslots // BLK_SLOTS):
                srcs = []
                for c in range(BLK_SLOTS):
                    s = b * BLK_SLOTS + c
                    srcs.append((zc_t[:, s * D : (s + 1) * D],
                                 zdc_t[:, s * D : (s + 1) * D]))
                _block(*blk, srcs, o_stage, b * BLK_SLOTS * P)
            nc.sync.dma_start(
                out=out_d[(obase + s0) * P : (obase + s0 + gslots) * P][None, :],
                in_=o_stage[:1, : gslots * P],
            )

    nc.compile()
    return nc


def _ragged_pack(vals, mask, width, fill=0):
    Pn, S = mask.shape
    out = np.full((Pn, width), fill, vals.dtype)
    cnt = mask.cumsum(1) - 1
    rows = np.broadcast_to(np.arange(Pn)[:, None], mask.shape)
    out[rows[mask], cnt[mask]] = vals[mask]
    return out


def pack_all(edge_label_index, e_core=E_CORE, s_lane=S_LANE):
    """Per-core (idx [P,3nA+2nC] int32, ORIG [P,2nA+nC] int64) + global nA,nC."""
    src_f = np.asarray(edge_label_index[0], dtype=np.int32)
    dst_f = np.asarray(edge_label_index[1], dtype=np.int32)
    ncores = len(src_f) // e_core
    cores = []
    for c in range(ncores):
        sl = slice(c * e_core, (c + 1) * e_core)
        s = np.zeros(s_lane * P, np.int32)
        t = np.zeros(s_lane * P, np.int32)
        s[:e_core] = src_f[sl]
        t[:e_core] = dst_f[sl]
        order = np.argsort(s, kind="stable").astype(np.int64)
        V = s[order].reshape(P, s_lane)
        Vd = t[order].reshape(P, s_lane)
        EO = order.reshape(P, s_lane)
        eq = V[:, 1:] == V[:, :-1]
        eqx = np.concatenate([eq, np.zeros((P, 1), bool)], 1)
        start = np.concatenate([np.ones((P, 1), bool), ~eq], 1)
        j = np.broadcast_to(np.arange(s_lane)[None, :], (P, s_lane))
        runstart = np.maximum.accumulate(np.where(start, j, 0), 1)
        pos = j - runstart
        pf = eqx & (pos % 2 == 0)
        ps = np.concatenate([np.zeros((P, 1), bool), pf[:, :-1]], 1)
        sg = ~pf & ~ps
        cores.append((V, Vd, EO, pf, ps, sg))
    nA = max(int(x[3].sum(1).max()) for x in cores)
    nC = max(int(x[5].sum(1).max()) for x in cores)
    nA = -(-nA // 2) * 2
    nC = -(-nC // BLK_SLOTS) * BLK_SLOTS
    packed = []
    for V, Vd, EO, pf, ps, sg in cores:
        idx = np.ascontiguousarray(np.concatenate([
            _ragged_pack(V, pf, nA), _ragged_pack(Vd, pf, nA),
            _ragged_pack(Vd, ps, nA), _ragged_pack(V, sg, nC),
            _ragged_pack(Vd, sg, nC)], axis=1))
        ORIG = np.full((P, 2 * nA + nC), -1, np.int64)
        ORIG[:, 0 : 2 * nA : 2] = _ragged_pack(EO, pf, nA, fill=-1)
        ORIG[:, 1 : 2 * nA : 2] = _ragged_pack(EO, ps, nA, fill=-1)
        ORIG[:, 2 * nA :] = _ragged_pack(EO, sg, nC, fill=-1)
        packed.append((idx, ORIG))
    return packed, nA, nC


_NC_CACHE = {}


def run(inputs, trace=False, **kw):
    z = np.ascontiguousarray(np.asarray(inputs["z"], dtype=np.float32))
    w1 = np.ascontiguousarray(np.asarray(inputs["W1"], dtype=np.float32))
    b1v = np.ascontiguousarray(np.asarray(inputs["b1"], dtype=np.float32))
    w2 = np.ascontiguousarray(np.asarray(inputs["W2"], dtype=np.float32))
    b2v = np.ascontiguousarray(np.asarray(inputs["b2"], dtype=np.float32))
    packed, nA, nC = pack_all(inputs["edge_label_index"])
    key = (nA, nC)
    if key not in _NC_CACHE:
        _NC_CACHE[key] = build_program(nA, nC)
    res = run_bass_kernel_spmd(
        _NC_CACHE[key],
        [{"z": z, "idx": idx, "w1": w1, "b1": b1v, "w2": w2, "b2": b2v}
         for idx, _ in packed],
        list(range(NCORES)), trace=trace, **kw)
    outs = []
    for c in range(NCORES):
        dev = res.results[c]["out"]
        orig_flat = packed[c][1].T.ravel()
        valid = orig_flat >= 0
        full = np.zeros(S_LANE * P, np.float32)
        full[orig_flat[valid]] = dev[valid]
        outs.append(full[:E_CORE])
    return np.concatenate(outs).astype(np.float32), res


def kernel(z, edge_label_index, W1, b1, W2, b2):
    out, _ = run({"z": z, "edge_label_index": edge_label_index,
                  "W1": W1, "b1": b1, "W2": W2, "b2": b2})
    return out



# revision 2
# speedup vs baseline: 1.0912x; 1.0912x over previous
"""HadamardMLPDecoder via SWDGE dma_gather on 4 queues.

Edges are grouped per-core by (src_window, dst_window) over 32768-row z
windows so every dma_gather instruction's int16 indices stay in-range.
Gathers land as [lane, slot, D]; compute = hadamard -> PE transpose ->
W1 matmul (fp32r) -> relu -> W2 matmul -> staged output. Host unpermutes.
"""

import numpy as np

import concourse.bass as bass
import concourse.mybir as mybir
import concourse.tile as tile
from concourse import bacc
from concourse.bass_utils import run_bass_kernel_spmd
from concourse.masks import make_identity
from contextlib import ExitStack

N, D, H = 100000, 128, 128
E_TOTAL = 2000000
NCORES = 8
P = 128
W = 32768  # z rows per int16 index window
BLK = 4    # slots per compute block
CHUNK = 8  # slots per dma_gather instruction (1024 descriptors)
E_CORE = E_TOTAL // NCORES

F32 = mybir.dt.float32
BF16 = mybir.dt.bfloat16
I16 = mybir.dt.int16
RELU = mybir.ActivationFunctionType.Relu
IDENT = mybir.ActivationFunctionType.Identity


def build_program(schedule, nslots):
    """schedule: tuple of (slot0, nslots_chunk, sw, dw) per gather chunk."""
    nc = bacc.Bacc("TRN2", target_bir_lowering=False, debug=False,
                   enable_asserts=False, num_devices=NCORES,
                   num_swdge_queues=4)
    z_d = nc.dram_tensor("z", [N, D], F32, kind="ExternalInput").ap()
    idx_d = nc.dram_tensor("idx", [P, 16 * nslots], I16,
                           kind="ExternalInput").ap()
    w1_d = nc.dram_tensor("w1", [D, H], F32, kind="ExternalInput").ap()
    b1_d = nc.dram_tensor("b1", [H], F32, kind="ExternalInput").ap()
    w2_d = nc.dram_tensor("w2", [H, 1], F32, kind="ExternalInput").ap()
    b2_d = nc.dram_tensor("b2", [1], F32, kind="ExternalInput").ap()
    out_d = nc.dram_tensor("out", [nslots * P], F32, kind="ExternalOutput").ap()

    wins = [z_d[w * W : min((w + 1) * W, N), :] for w in range(4)]

    with tile.TileContext(nc) as tc, ExitStack() as ctx:
        const = ctx.enter_context(tc.tile_pool(name="const", bufs=1))
        zpool = ctx.enter_context(tc.tile_pool(name="gather", bufs=3))
        work = ctx.enter_context(tc.tile_pool(name="work", bufs=3))
        stage_pool = ctx.enter_context(tc.tile_pool(name="stage", bufs=3))
        psum_t = ctx.enter_context(tc.tile_pool(name="ps_t", bufs=2, space="PSUM"))
        psum_h = ctx.enter_context(tc.tile_pool(name="ps_h", bufs=2, space="PSUM"))
        psum_o = ctx.enter_context(tc.tile_pool(name="ps_o", bufs=2, space="PSUM"))

        idx_sb = const.tile([P, 16 * nslots], I16)
        nc.sync.dma_start(out=idx_sb[:], in_=idx_d[:, :])
        w1_sb = const.tile([P, H], F32)
        nc.sync.dma_start(out=w1_sb[:], in_=w1_d[:, :])
        b1_sb = const.tile([P, 1], F32)
        nc.sync.dma_start(out=b1_sb[:], in_=b1_d[:, None])
        w2_sb = const.tile([P, 1], F32)
        nc.sync.dma_start(out=w2_sb[:], in_=w2_d[:, :])
        b2_sb = const.tile([1, 1], F32)
        nc.sync.dma_start(out=b2_sb[:1], in_=b2_d[:, None])
        ident = const.tile([P, P], BF16)
        make_identity(nc, ident[:])
        w1_bf = const.tile([P, H], BF16)
        nc.vector.tensor_copy(out=w1_bf[:], in_=w1_sb[:])
        w2_bf = const.tile([P, 1], BF16)
        nc.vector.tensor_copy(out=w2_bf[:], in_=w2_sb[:])

        qn = [0]

        def gather(dst_ap, win, col0, cols, nidx):
            nc.gpsimd.dma_gather(
                dst_ap, wins[win], idx_sb[:, col0 : col0 + cols],
                nidx, nidx, D, queue_num=qn[0] % 4,
            )
            qn[0] += 1

        dcol0 = 8 * nslots  # dst idx cols start after all src cols
        for (s0, ns, sw, dw) in schedule:
            zs = zpool.tile([P, CHUNK, D], F32, tag="zs")
            zd = zpool.tile([P, CHUNK, D], F32, tag="zd")
            gather(zs[:, :ns, :], sw, 8 * s0, 8 * ns, ns * P)
            gather(zd[:, :ns, :], dw, dcol0 + 8 * s0, 8 * ns, ns * P)
            o_stage = stage_pool.tile([1, CHUNK * P], F32, tag="ostage")
            for b in range(ns // BLK):
                e0 = b * BLK * P
                EB = BLK * P
                ef = work.tile([P, BLK, D], BF16, tag="ef")
                nc.vector.tensor_mul(out=ef[:, :, :],
                                     in0=zs[:, b * BLK : (b + 1) * BLK, :],
                                     in1=zd[:, b * BLK : (b + 1) * BLK, :])
                efT_ps = psum_t.tile([P, EB], BF16)
                for c in range(BLK):
                    nc.tensor.transpose(
                        out=efT_ps[:, c * P : (c + 1) * P],
                        in_=ef[:, c, :],
                        identity=ident[:],
                    )
                efT = work.tile([P, EB], BF16, tag="efT")
                nc.vector.tensor_copy(out=efT[:], in_=efT_ps[:])
                h_ps = psum_h.tile([P, EB], F32)
                nc.tensor.matmul(out=h_ps[:], lhsT=w1_bf[:],
                                 rhs=efT[:], start=True, stop=True)
                h_sb = work.tile([P, EB], BF16, tag="h")
                nc.scalar.activation(out=h_sb[:], in_=h_ps[:], func=RELU,
                                     bias=b1_sb[:, :1], scale=1.0)
                o_ps = psum_o.tile([1, EB], F32)
                nc.tensor.matmul(out=o_ps[:], lhsT=w2_bf[:],
                                 rhs=h_sb[:], start=True, stop=True)
                nc.scalar.activation(
                    out=o_stage[:1, e0 : e0 + EB], in_=o_ps[:], func=IDENT,
                    bias=b2_sb[:1, :1], scale=1.0,
                )
            nc.sync.dma_start(
                out=out_d[s0 * P : (s0 + ns) * P][None, :],
                in_=o_stage[:1, : ns * P],
            )

    nc.compile()
    return nc


def _wrap(flat):
    """flat [S*128] -> wrapped [128, S*8] int16 (idx i at [i%16, i//16],
    16-partition block replicated x8)."""
    w = flat.reshape(-1, 16).T.astype(np.int16)
    return np.tile(w, (8, 1))


def pack_all(edge_label_index):
    """Per-core (idx [128, 16*S] int16, orig [S*128] int64); shared schedule.

    Groups edges by gid=(src_win*4+dst_win); pads every group to the max
    slot count across cores (so all 8 cores share one program/schedule)."""
    src_f = np.asarray(edge_label_index[0], dtype=np.int64)
    dst_f = np.asarray(edge_label_index[1], dtype=np.int64)
    cores = []
    for c in range(NCORES):
        sl = slice(c * E_CORE, (c + 1) * E_CORE)
        s, d = src_f[sl], dst_f[sl]
        orig = np.arange(c * E_CORE, (c + 1) * E_CORE, dtype=np.int64)
        g = (s // W) * 4 + (d // W)
        order = np.argsort(g, kind="stable")
        cores.append((s[order], d[order], orig[order], g[order]))
    # slots per gid = max over cores, rounded up to BLK
    nsl = {}
    for gid in range(16):
        mx = max(int((g == gid).sum()) for _, _, _, g in cores)
        slots = -(-mx // P)
        slots = -(-slots // BLK) * BLK
        nsl[gid] = slots
    S = sum(nsl.values())
    schedule = []
    base = 0
    for gid in range(16):
        sw, dw = gid // 4, gid % 4
        r = 0
        while r < nsl[gid]:
            ns = min(CHUNK, nsl[gid] - r)
            schedule.append((base + r, ns, sw, dw))
            r += ns
        base += nsl[gid]
    out = []
    for (s, d, orig, g) in cores:
        gs_all, gd_all, go_all = [], [], []
        for gid in range(16):
            sw, dw = gid // 4, gid % 4
            m = g == gid
            gs, gd, go = s[m], d[m], orig[m]
            pad = nsl[gid] * P - len(gs)
            assert pad >= 0
            if pad:
                gs = np.concatenate([gs, np.full(pad, sw * W, np.int64)])
                gd = np.concatenate([gd, np.full(pad, dw * W, np.int64)])
                go = np.concatenate([go, np.full(pad, -1, np.int64)])
            gs_all.append(gs - sw * W)
            gd_all.append(gd - dw * W)
            go_all.append(go)
        idx16 = np.concatenate([_wrap(np.concatenate(gs_all)),
                                _wrap(np.concatenate(gd_all))], axis=1)
        out.append((np.ascontiguousarray(idx16), np.concatenate(go_all)))
    return out, tuple(schedule), S


_NC_CACHE = {}


def run(inputs, trace=False, **kw):
    z = np.ascontiguousarray(np.asarray(inputs["z"], dtype=np.float32))
    w1 = np.ascontiguousarray(np.asarray(inputs["W1"], dtype=np.float32))
    b1v = np.ascontiguousarray(np.asarray(inputs["b1"], dtype=np.float32))
    w2 = np.ascontiguousarray(np.asarray(inputs["W2"], dtype=np.float32))
    b2v = np.ascontiguousarray(np.asarray(inputs["b2"], dtype=np.float32))
    packed, sched, S = pack_all(inputs["edge_label_index"])
    key = (sched, S)
    if key not in _NC_CACHE:
        _NC_CACHE[key] = build_program(sched, S)
    res = run_bass_kernel_spmd(
        _NC_CACHE[key],
        [{"z": z, "idx": idx, "w1": w1, "b1": b1v, "w2": w2, "b2": b2v}
         for idx, _ in packed],
        list(range(NCORES)), trace=trace, **kw)
    outs = np.zeros(E_TOTAL, np.float32)
    for c in range(NCORES):
        dev = res.results[c]["out"]
        orig = packed[c][1]
        valid = orig >= 0
        outs[orig[valid]] = dev[valid]
    return outs, res


def kernel(z, edge_label_index, W1, b1, W2, b2):
    out, _ = run({"z": z, "edge_label_index": edge_label_index,
                  "W1": W1, "b1": b1, "W2": W2, "b2": b2})
    return out


# revision 4
# speedup vs baseline: 1.3234x; 1.2128x over previous
"""HadamardMLPDecoder: 4-queue SWDGE dma_gather + src pair-sharing.

Within each core, edges with equal src are paired so two edges share one
src-row descriptor (phase A: superslots). Remaining edges are singles
(phase C). Groups are keyed by z-row windows (32768 rows, int16 indices);
-1 index tail-padding makes pad descriptors free. Compute: hadamard ->
PE transpose (bf16) -> W1 matmul -> relu -> W2 matmul. Host unpermutes.
"""

import numpy as np

import concourse.bass as bass
import concourse.mybir as mybir
import concourse.tile as tile
from concourse import bacc
from concourse.bass_utils import run_bass_kernel_spmd
from concourse.masks import make_identity
from contextlib import ExitStack

N, D, H = 100000, 128, 128
E_TOTAL = 2000000
NCORES = 8
P = 128
W = 32768
BLK = 4     # slots per compute block
CHUNK = 8   # slots (or superslots) per dma_gather chunk
E_CORE = E_TOTAL // NCORES

F32 = mybir.dt.float32
BF16 = mybir.dt.bfloat16
I16 = mybir.dt.int16
RELU = mybir.ActivationFunctionType.Relu
IDENT = mybir.ActivationFunctionType.Identity


def build_program(schedA, schedC, nsup, nsing):
    """schedA: (sup0, nsu, sw, wA, wB, nvS, nvA, nvB) chunks over superslots.
    schedC: (sl0, ns, sw, dw, nvS, nvD) chunks over single slots.
    idx tensor column sections (int16, wrapped 8 cols/slot):
      [srcA: nsup] [dstA: nsup] [dstB: nsup] [srcC: nsing] [dstC: nsing]
    out: [(2*nsup + nsing) * 128] f32; superslot w -> out slots 2w, 2w+1.
    """
    nc = bacc.Bacc("TRN2", target_bir_lowering=False, debug=False,
                   enable_asserts=False, num_devices=NCORES,
                   num_swdge_queues=4)
    ncols = 8 * (3 * nsup + 2 * nsing)
    z_d = nc.dram_tensor("z", [N, D], F32, kind="ExternalInput").ap()
    idx_d = nc.dram_tensor("idx", [P, ncols], I16, kind="ExternalInput").ap()
    w1_d = nc.dram_tensor("w1", [D, H], F32, kind="ExternalInput").ap()
    b1_d = nc.dram_tensor("b1", [H], F32, kind="ExternalInput").ap()
    w2_d = nc.dram_tensor("w2", [H, 1], F32, kind="ExternalInput").ap()
    b2_d = nc.dram_tensor("b2", [1], F32, kind="ExternalInput").ap()
    out_d = nc.dram_tensor("out", [(2 * nsup + nsing) * P], F32,
                           kind="ExternalOutput").ap()

    wins = [z_d[w * W : min((w + 1) * W, N), :] for w in range(4)]

    with tile.TileContext(nc) as tc, ExitStack() as ctx:
        const = ctx.enter_context(tc.tile_pool(name="const", bufs=1))
        zpool = ctx.enter_context(tc.tile_pool(name="gather", bufs=3))
        work = ctx.enter_context(tc.tile_pool(name="work", bufs=3))
        stage_pool = ctx.enter_context(tc.tile_pool(name="stage", bufs=3))
        psum_t = ctx.enter_context(tc.tile_pool(name="ps_t", bufs=2, space="PSUM"))
        psum_h = ctx.enter_context(tc.tile_pool(name="ps_h", bufs=2, space="PSUM"))
        psum_o = ctx.enter_context(tc.tile_pool(name="ps_o", bufs=2, space="PSUM"))

        idx_sb = const.tile([P, ncols], I16)
        nc.sync.dma_start(out=idx_sb[:], in_=idx_d[:, :])
        w1_sb = const.tile([P, H], F32)
        nc.sync.dma_start(out=w1_sb[:], in_=w1_d[:, :])
        b1_sb = const.tile([P, 1], F32)
        nc.sync.dma_start(out=b1_sb[:], in_=b1_d[:, None])
        w2_sb = const.tile([P, 1], F32)
        nc.sync.dma_start(out=w2_sb[:], in_=w2_d[:, :])
        b2_sb = const.tile([1, 1], F32)
        nc.sync.dma_start(out=b2_sb[:1], in_=b2_d[:, None])
        ident = const.tile([P, P], BF16)
        make_identity(nc, ident[:])
        w1_bf = const.tile([P, H], BF16)
        nc.vector.tensor_copy(out=w1_bf[:], in_=w1_sb[:])
        w2_bf = const.tile([P, 1], BF16)
        nc.vector.tensor_copy(out=w2_bf[:], in_=w2_sb[:])

        qn = [0]

        def gather(dst_ap, win, col0, cols, nidx, nvalid):
            nc.gpsimd.dma_gather(
                dst_ap, wins[win], idx_sb[:, col0 : col0 + cols],
                nidx, nvalid, D, queue_num=qn[0] % 4,
            )
            qn[0] += 1

        def mlp_block(ef, o_stage, e0):
            """ef: [P, BLK, D] bf16 tile -> outputs staged at o_stage[e0:]."""
            EB = BLK * P
            efT_ps = psum_t.tile([P, EB], BF16)
            for c in range(BLK):
                nc.tensor.transpose(
                    out=efT_ps[:, c * P : (c + 1) * P],
                    in_=ef[:, c, :], identity=ident[:],
                )
            efT = work.tile([P, EB], BF16, tag="efT")
            nc.vector.tensor_copy(out=efT[:], in_=efT_ps[:])
            h_ps = psum_h.tile([P, EB], F32)
            nc.tensor.matmul(out=h_ps[:], lhsT=w1_bf[:], rhs=efT[:],
                             start=True, stop=True)
            h_sb = work.tile([P, EB], BF16, tag="h")
            nc.scalar.activation(out=h_sb[:], in_=h_ps[:], func=RELU,
                                 bias=b1_sb[:, :1], scale=1.0)
            o_ps = psum_o.tile([1, EB], F32)
            nc.tensor.matmul(out=o_ps[:], lhsT=w2_bf[:], rhs=h_sb[:],
                             start=True, stop=True)
            nc.scalar.activation(
                out=o_stage[:1, e0 : e0 + EB], in_=o_ps[:], func=IDENT,
                bias=b2_sb[:1, :1], scale=1.0,
            )

        cS, cA, cB = 0, 8 * nsup, 16 * nsup
        # ---- phase A: superslots (one src cell serves two dst slots) ----
        for (u0, nu, sw, wA, wB, nvS, nvA, nvB) in schedA:
            zs = zpool.tile([P, CHUNK, D], F32, tag="zs")
            zda = zpool.tile([P, CHUNK, D], F32, tag="zda")
            zdb = zpool.tile([P, CHUNK, D], F32, tag="zdb")
            gather(zs[:, :nu, :], sw, cS + 8 * u0, 8 * nu, nu * P, nvS)
            gather(zda[:, :nu, :], wA, cA + 8 * u0, 8 * nu, nu * P, nvA)
            gather(zdb[:, :nu, :], wB, cB + 8 * u0, 8 * nu, nu * P, nvB)
            o_stage = stage_pool.tile([1, 2 * CHUNK * P], F32, tag="ostage")
            for b in range(nu // 2):  # block = 2 superslots -> 4 out slots
                w0 = b * 2
                ef = work.tile([P, BLK, D], BF16, tag="ef")
                nc.vector.tensor_mul(
                    out=ef[:, 0:2, :], in0=zs[:, w0 : w0 + 2, :],
                    in1=zda[:, w0 : w0 + 2, :])
                nc.vector.tensor_mul(
                    out=ef[:, 2:4, :], in0=zs[:, w0 : w0 + 2, :],
                    in1=zdb[:, w0 : w0 + 2, :])
                mlp_block(ef, o_stage, b * BLK * P)
            nc.sync.dma_start(
                out=out_d[(2 * u0) * P : (2 * u0 + 2 * nu) * P][None, :],
                in_=o_stage[:1, : 2 * nu * P],
            )

        # ---- phase C: single slots ----
        cSs, cDs = 24 * nsup, 24 * nsup + 8 * nsing
        obase = 2 * nsup
        for (s0, ns, sw, dw, nvS, nvD) in schedC:
            zs = zpool.tile([P, CHUNK, D], F32, tag="zs")
            zd = zpool.tile([P, CHUNK, D], F32, tag="zda")
            gather(zs[:, :ns, :], sw, cSs + 8 * s0, 8 * ns, ns * P, nvS)
            gather(zd[:, :ns, :], dw, cDs + 8 * s0, 8 * ns, ns * P, nvD)
            o_stage = stage_pool.tile([1, 2 * CHUNK * P], F32, tag="ostage")
            for b in range(ns // BLK):
                ef = work.tile([P, BLK, D], BF16, tag="ef")
                nc.vector.tensor_mul(out=ef[:, :, :],
                                     in0=zs[:, b * BLK : (b + 1) * BLK, :],
                                     in1=zd[:, b * BLK : (b + 1) * BLK, :])
                mlp_block(ef, o_stage, b * BLK * P)
            nc.sync.dma_start(
                out=out_d[(obase + s0) * P : (obase + s0 + ns) * P][None, :],
                in_=o_stage[:1, : ns * P],
            )

    nc.compile()
    return nc


def _wrap(flat):
    w = flat.reshape(-1, 16).T.astype(np.int16)
    return np.tile(w, (8, 1))


def pack_all(edge_label_index):
    src_f = np.asarray(edge_label_index[0], dtype=np.int64)
    dst_f = np.asarray(edge_label_index[1], dtype=np.int64)
    cores = []
    for c in range(NCORES):
        sl = slice(c * E_CORE, (c + 1) * E_CORE)
        s, d = src_f[sl], dst_f[sl]
        orig = np.arange(c * E_CORE, (c + 1) * E_CORE, dtype=np.int64)
        order = np.argsort(s, kind="stable")
        s, d, orig = s[order], d[order], orig[order]
        # pair consecutive same-src edges
        first = np.ones(len(s), bool)
        first[1:] = s[1:] != s[:-1]
        runpos = np.arange(len(s)) - np.maximum.accumulate(
            np.where(first, np.arange(len(s)), 0))
        isA = (runpos % 2 == 0)
        nxt_same = np.zeros(len(s), bool)
        nxt_same[:-1] = s[1:] == s[:-1]
        is_pairA = isA & nxt_same
        is_pairB = np.zeros(len(s), bool)
        is_pairB[1:] = is_pairA[:-1]
        is_single = ~is_pairA & ~is_pairB
        pa = np.flatnonzero(is_pairA)
        # pairs: shared src s[pa], dsts d[pa], d[pa+1]
        ps, da, db = s[pa], d[pa], d[pa + 1]
        oa, ob = orig[pa], orig[pa + 1]
        # canonical window order
        swp = np.minimum(da // W, db // W)
        swq = np.maximum(da // W, db // W)
        flip = (da // W) > (db // W)
        da2 = np.where(flip, db, da)
        db2 = np.where(flip, da, db)
        oa2 = np.where(flip, ob, oa)
        ob2 = np.where(flip, oa, ob)
        gidA = (ps // W) * 16 + swp * 4 + swq
        si = np.flatnonzero(is_single)
        gidC = (s[si] // W) * 4 + d[si] // W
        cores.append(((ps, da2, db2, oa2, ob2, gidA),
                      (s[si], d[si], orig[si], gidC)))
    # budgets per gid (max over cores), in superslots / slots
    gidsA = sorted(set(int(g) for core in cores for g in core[0][5]))
    gidsC = sorted(set(int(g) for core in cores for g in core[1][3]))
    budA = {g: max(-(-int((core[0][5] == g).sum()) // P) for core in cores)
            for g in gidsA}
    budC = {g: max(-(-int((core[1][3] == g).sum()) // P) for core in cores)
            for g in gidsC}
    # round up so every group is a multiple of 2 superslots / BLK slots
    for g in budA:
        budA[g] = -(-budA[g] // 2) * 2
    for g in budC:
        budC[g] = -(-budC[g] // BLK) * BLK
    nsup = sum(budA.values())
    nsing = sum(budC.values())

    # schedules (shared), with per-chunk valid counts = max over cores so the
    # ucode processes every core's valid descriptors (extra -1s are skipped
    # only if count says so -> use per-core counts? ucode asserts
    # num_idxs_reg == nonneg count, so counts must match per core exactly.
    # Simplest: make counts equal across cores by padding with window-base
    # index (0) instead of -1 up to the max-valid count, -1 beyond.
    schedA, schedC = [], []
    base = 0
    for g in gidsA:
        r = 0
        while r < budA[g]:
            nu = min(CHUNK, budA[g] - r)
            schedA.append([base + r, nu, g // 16, (g // 4) % 4, g % 4])
            r += nu
        base += budA[g]
    base = 0
    for g in gidsC:
        r = 0
        while r < budC[g]:
            ns = min(CHUNK, budC[g] - r)
            schedC.append([base + r, ns, g // 4, g % 4])
            r += ns
        base += budC[g]

    # per-core packing + per-chunk valid counts (must be uniform -> compute
    # per-core count per chunk, take max, and pad shorter cores with index 0
    # (valid, gathers window base row) so counts line up.
    nvalA = np.zeros((len(schedA), 3), np.int64)  # src, dstA, dstB
    nvalC = np.zeros((len(schedC), 2), np.int64)
    packedA, packedC = [], []
    for core in cores:
        (ps, da, db, oa, ob, gidA), (ss, sd, so, gidC) = core
        sA = np.full(nsup * P, -1, np.int64)
        dA = np.full(nsup * P, -1, np.int64)
        dB = np.full(nsup * P, -1, np.int64)
        oA = np.full(nsup * P, -1, np.int64)
        oB = np.full(nsup * P, -1, np.int64)
        basec = 0
        for g in gidsA:
            m = gidA == g
            k = int(m.sum())
            o0 = basec * P
            sA[o0 : o0 + k] = ps[m] - (g // 16) * W
            dA[o0 : o0 + k] = da[m] - ((g // 4) % 4) * W
            dB[o0 : o0 + k] = db[m] - (g % 4) * W
            oA[o0 : o0 + k] = oa[m]
            oB[o0 : o0 + k] = ob[m]
            basec += budA[g]
        sC = np.full(nsing * P, -1, np.int64)
        dC = np.full(nsing * P, -1, np.int64)
        oC = np.full(nsing * P, -1, np.int64)
        basec = 0
        for g in gidsC:
            m = gidC == g
            k = int(m.sum())
            o0 = basec * P
            sC[o0 : o0 + k] = ss[m] - (g // 4) * W
            dC[o0 : o0 + k] = sd[m] - (g % 4) * W
            oC[o0 : o0 + k] = so[m]
            basec += budC[g]
        packedA.append((sA, dA, dB, oA, oB))
        packedC.append((sC, dC, oC))
        for i, (u0, nu, *_rest) in enumerate(schedA):
            blk = slice(u0 * P, (u0 + nu) * P)
            nvalA[i, 0] = max(nvalA[i, 0], int((sA[blk] >= 0).sum()))
            nvalA[i, 1] = max(nvalA[i, 1], int((dA[blk] >= 0).sum()))
            nvalA[i, 2] = max(nvalA[i, 2], int((dB[blk] >= 0).sum()))
        for i, (s0, ns, *_rest) in enumerate(schedC):
            blk = slice(s0 * P, (s0 + ns) * P)
            nvalC[i, 0] = max(nvalC[i, 0], int((sC[blk] >= 0).sum()))
            nvalC[i, 1] = max(nvalC[i, 1], int((dC[blk] >= 0).sum()))

    # pad shorter cores to the max valid count with index 0
    out = []
    for (sA, dA, dB, oA, oB), (sC, dC, oC) in zip(packedA, packedC):
        for i, (u0, nu, *_r) in enumerate(schedA):
            for arr, col in ((sA, 0), (dA, 1), (dB, 2)):
                blk = arr[u0 * P : (u0 + nu) * P]
                need = int(nvalA[i, col]) - int((blk >= 0).sum())
                if need > 0:
                    fill = np.flatnonzero(blk < 0)[:need]
                    blk[fill] = 0
        for i, (s0, ns, *_r) in enumerate(schedC):
            for arr, col in ((sC, 0), (dC, 1)):
                blk = arr[s0 * P : (s0 + ns) * P]
                need = int(nvalC[i, col]) - int((blk >= 0).sum())
                if need > 0:
                    fill = np.flatnonzero(blk < 0)[:need]
                    blk[fill] = 0
        idx16 = np.concatenate(
            [_wrap(sA), _wrap(dA), _wrap(dB), _wrap(sC), _wrap(dC)], axis=1)
        # out-slot order per block b (2 superslots w0=2b, w0+1):
        # (A_w0, A_w0+1, B_w0, B_w0+1)
        oA2 = oA.reshape(nsup // 2, 2, P)
        oB2 = oB.reshape(nsup // 2, 2, P)
        origA = np.concatenate([oA2, oB2], axis=1).reshape(-1)
        out.append((np.ascontiguousarray(idx16),
                    np.concatenate([origA, oC])))

    schedA_t = tuple((u0, nu, sw, wa, wb, int(nvalA[i, 0]), int(nvalA[i, 1]),
                      int(nvalA[i, 2]))
                     for i, (u0, nu, sw, wa, wb) in enumerate(schedA))
    schedC_t = tuple((s0, ns, sw, dw, int(nvalC[i, 0]), int(nvalC[i, 1]))
                     for i, (s0, ns, sw, dw) in enumerate(schedC))
    return out, schedA_t, schedC_t, nsup, nsing


_NC_CACHE = {}


def run(inputs, trace=False, **kw):
    z = np.ascontiguousarray(np.asarray(inputs["z"], dtype=np.float32))
    w1 = np.ascontiguousarray(np.asarray(inputs["W1"], dtype=np.float32))
    b1v = np.ascontiguousarray(np.asarray(inputs["b1"], dtype=np.float32))
    w2 = np.ascontiguousarray(np.asarray(inputs["W2"], dtype=np.float32))
    b2v = np.ascontiguousarray(np.asarray(inputs["b2"], dtype=np.float32))
    packed, schedA, schedC, nsup, nsing = pack_all(inputs["edge_label_index"])
    key = (schedA, schedC, nsup, nsing)
    if key not in _NC_CACHE:
        _NC_CACHE[key] = build_program(schedA, schedC, nsup, nsing)
    res = run_bass_kernel_spmd(
        _NC_CACHE[key],
        [{"z": z, "idx": idx, "w1": w1, "b1": b1v, "w2": w2, "b2": b2v}
         for idx, _ in packed],
        list(range(NCORES)), trace=trace, **kw)
    outs = np.zeros(E_TOTAL, np.float32)
    for c in range(NCORES):
        dev = res.results[c]["out"]
        orig = packed[c][1]
        valid = orig >= 0
        outs[orig[valid]] = dev[valid]
    return outs, res


def kernel(z, edge_label_index, W1, b1, W2, b2):
    out, _ = run({"z": z, "edge_label_index": edge_label_index,
                  "W1": W1, "b1": b1, "W2": W2, "b2": b2})
    return out
